# revision 1
# baseline (speedup 1.0000x reference)
"""LocallyConnected2d Trainium2 kernel.

Problem: out[b,o,h,w] = sum_{c,i,j} xpad[b,c,h+i,w+j] * weights[h,w,o,c,i,j] + bias[o,h,w]
  B=32, C=32, O=32, H=W=64, K=3, PAD=1, OH=OW=64.

Sharding: each of the 8 cores owns a band of 8 output rows (OH split), with the
matching 10-row input halo. Weights (the dominant traffic, 144 MiB) split 1/8
per core with zero redundancy.

Device compute: per output location (h,w) the contraction over (c,i,j)=288 is
split into 3 matmuls of K=96 (tap row i fixed, contraction over (c,j)),
accumulated in PSUM. M=o=32, N=b=32. The 4 locations of an ow-group share the
PE array via col-group tile_position packing. All layout transforms (getting
(c,j) onto partitions for both operands) are done host-side in numpy so every
DMA is contiguous.

Host-prepped per-core layouts:
  xp [96, 10*64*32] : [(c,j), r', w, b] = xpad[b, c, 8d+r', w+j]  (x replicated 3x, j-shifted)
  wp [8, 96, 16*384]: [h, (c,j), g, i, w4, o] = weights[8d+h, 4g+w4, o, c, i, j]
  bp [128, 8*16]    : [(w4,o), h, g] = bias[o, 8d+h, 4g+w4]
  op [8, 128, 512]  : [h, (w4,o), g, b]  (output)
"""

import sys

if "/opt/trn_rl_repo" not in sys.path:
    sys.path.insert(0, "/opt/trn_rl_repo")

import numpy as np

B = 32
C = 32
O = 32
H = W = 64
KK = 3
NCORES = 8
RP = H // NCORES      # output rows per core
RIN = RP + KK - 1     # input rows incl halo
P = 96                # contraction partitions (c,j)
NG = W // 4           # ow groups of 4
XGROUPS = [(0, 3), (3, 3), (6, 4)]          # x load groups (start row, nrows)

_built = {}


def _build():
    if "nc" in _built:
        return _built["nc"]
    import concourse.tile as tile
    from concourse.tile import add_dep_helper
    from concourse import bacc, mybir

    nc = bacc.Bacc("TRN2", target_bir_lowering=False, debug=False,
                   num_devices=NCORES)
    xp = nc.dram_tensor("xp", [P, RIN * W * B], mybir.dt.float32,
                        kind="ExternalInput")
    wp = nc.dram_tensor("wp", [P, RP * NG * 384], mybir.dt.float32,
                        kind="ExternalInput")
    bp = nc.dram_tensor("bp", [128, RP * NG], mybir.dt.float32,
                        kind="ExternalInput")
    op = nc.dram_tensor("op", [RP // 2, 128, 2 * NG * 32], mybir.dt.float32,
                        kind="ExternalOutput")

    f32 = mybir.dt.float32
    HF = NG * 384        # free elems per h row in wp
    with tile.TileContext(nc) as tc:
        with tc.tile_pool(name="xpool", bufs=1) as xpool, \
             tc.tile_pool(name="wpool", bufs=4) as wpool, \
             tc.tile_pool(name="opool", bufs=2) as opool, \
             tc.tile_pool(name="cpool", bufs=1) as cpool, \
             tc.tile_pool(name="ppool", bufs=4, space="PSUM") as ppool:
            loads = []

            def chain(inst):
                # keep at most ~2 load DMAs in flight: each waits for the
                # load two before it to complete
                if len(loads) >= 2:
                    add_dep_helper(inst.ins, loads[-2].ins, sync=True,
                                   reason="load chain")
                loads.append(inst)

            bt = cpool.tile([128, RP * NG], f32, tag="bias")
            chain(nc.sync.dma_start(bt[:], bp.ap()))
            xg = []
            rowtile = {}
            for gi, (r0, nr) in enumerate(XGROUPS):
                t = xpool.tile([P, nr * W * B], f32, tag=f"xg{gi}")
                xg.append(t)
                for r in range(r0, r0 + nr):
                    rowtile[r] = (t, (r - r0) * W * B)

            def load_xg(gi):
                r0, nr = XGROUPS[gi]
                chain(nc.sync.dma_start(
                    xg[gi][:], xp.ap()[:, r0 * W * B:(r0 + nr) * W * B]))

            def load_w(h):
                t = wpool.tile([P, HF], f32, tag="w")
                chain(nc.sync.dma_start(t[:], wp.ap()[:, h * HF:(h + 1) * HF]))
                return t

            load_xg(0)
            wq = [load_w(0), load_w(1)]
            load_xg(1)
            wq.append(load_w(2))
            load_xg(2)
            ot = None
            for h in range(RP):
                wth = wq.pop(0)
                ps = ppool.tile([128, NG * 32], f32, tag="ps")
                for g in range(NG):
                    for w4 in range(4):
                        wo = g * 384 + w4 * 32
                        xo = (4 * g + w4) * 32
                        for i in range(KK):
                            t, base = rowtile[h + i]
                            nc.tensor.matmul(
                                ps[32 * w4:32 * w4 + 32, 32 * g:32 * g + 32],
                                wth[:, wo + i * 128:wo + i * 128 + 32],
                                t[:, base + xo:base + xo + 32],
                                start=(i == 0),
                                stop=(i == KK - 1),
                                tile_position=(0, 32 * w4),
                            )
                if h + 3 < RP:
                    wq.append(load_w(h + 3))
                if h % 2 == 0:
                    ot = opool.tile([128, 2 * NG * 32], f32, tag="o")
                off = (h % 2) * NG * 32
                for g in range(NG):
                    nc.vector.tensor_scalar_add(
                        ot[:, off + 32 * g:off + 32 * g + 32],
                        ps[:, 32 * g:32 * g + 32],
                        bt[:, h * NG + g:h * NG + g + 1],
                    )
                if h % 2 == 1:
                    nc.scalar.dma_start(op.ap()[h // 2], ot[:])
    nc.compile()
    _built["nc"] = nc
    return nc


def prep_inputs(x, weights, bias):
    """Host-side shard + layout prep. Returns list of 8 in_maps."""
    x = np.asarray(x, dtype=np.float32)
    weights = np.asarray(weights, dtype=np.float32)
    bias = np.asarray(bias, dtype=np.float32)
    xpad = np.zeros((B, C, H + 2, W + 2), dtype=np.float32)
    xpad[:, :, 1:H + 1, 1:W + 1] = x
    in_maps = []
    for d in range(NCORES):
        blk = xpad[:, :, RP * d:RP * d + RIN, :]          # [b, c, 10, 66]
        xprep = np.empty((C, KK, RIN, W, B), dtype=np.float32)
        for j in range(KK):
            xprep[:, j] = blk[:, :, :, j:j + W].transpose(1, 2, 3, 0)
        xprep = xprep.reshape(P, RIN * W * B)

        wd = weights[RP * d:RP * d + RP]                  # [8, 64, 32, 32, 3, 3]
        wd = wd.reshape(RP, NG, 4, O, C, KK, KK)          # h, g, w4, o, c, i, j
        wcj = wd.transpose(4, 6, 0, 1, 5, 2, 3)           # c, j, h, g, i, w4, o
        wprep = np.ascontiguousarray(wcj).reshape(P, RP * NG * 384)

        bd = bias[:, RP * d:RP * d + RP, :].reshape(O, RP, NG, 4)
        bprep = np.ascontiguousarray(bd.transpose(3, 0, 1, 2)).reshape(
            128, RP * NG)                                  # (w4,o), (h,g)
        in_maps.append({"xp": xprep, "wp": wprep, "bp": bprep})
    return in_maps


def assemble_output(results):
    """results: list of 8 dicts with 'op' [4, 128, 1024] -> full [B,O,H,W]."""
    out = np.empty((B, O, H, W), dtype=np.float32)
    for d in range(NCORES):
        arr = np.asarray(results[d]["op"]).reshape(RP // 2, 4, O, 2, NG, B)
        # [ck, w4, o, hh, g, b] -> [b, o, (ck,hh), g, w4]
        out[:, :, RP * d:RP * d + RP, :] = (
            arr.transpose(5, 2, 0, 3, 4, 1).reshape(B, O, RP, W))
    return out


def _ensure_ntff_hook():
    """The agent image's antenv lacks axon_hooks; inject it and register the
    ctypes NTFF hook (same recipe as trn_agent_boot.trn_boot)."""
    try:
        from antenv.axon_hooks import get_axon_ntff_profile_hook  # noqa: F401
        return
    except ImportError:
        pass
    import types
    import ctypes
    import contextlib

    mod = types.ModuleType("antenv.axon_hooks")
    mod._hook = None

    def set_axon_ntff_profile_hook(h):
        mod._hook = h

    def get_axon_ntff_profile_hook():
        return mod._hook

    mod.set_axon_ntff_profile_hook = set_axon_ntff_profile_hook
    mod.get_axon_ntff_profile_hook = get_axon_ntff_profile_hook
    sys.modules["antenv.axon_hooks"] = mod
    import antenv

    antenv.axon_hooks = mod

    so_path = "/opt/axon/libaxon_pjrt.so"
    try:
        lib = ctypes.CDLL(so_path)
    except OSError:
        return
    if not hasattr(lib, "axon_start_nrt_profile"):
        return
    lib.axon_start_nrt_profile.argtypes = [
        ctypes.POINTER(ctypes.c_int64), ctypes.c_size_t]
    lib.axon_start_nrt_profile.restype = ctypes.c_int64
    lib.axon_stop_nrt_profile.argtypes = [ctypes.c_char_p]
    lib.axon_stop_nrt_profile.restype = ctypes.c_int64

    @contextlib.contextmanager
    def _hook(output_dir, device_ids):
        import jax

        jax.devices()
        if device_ids:
            ids = (ctypes.c_int64 * len(device_ids))(*device_ids)
            rc = lib.axon_start_nrt_profile(ids, len(device_ids))
        else:
            rc = lib.axon_start_nrt_profile(None, 0)
        if rc != 0:
            raise RuntimeError(f"axon_start_nrt_profile rc={rc}")
        try:
            yield
        finally:
            n = lib.axon_stop_nrt_profile(str(output_dir).encode())
            print(f"ntff profile: {n} file(s) written to {output_dir}")

    mod.set_axon_ntff_profile_hook(_hook)


def run(inputs, trace=False, **kwargs):
    from concourse.bass_utils import run_bass_kernel_spmd

    if trace:
        _ensure_ntff_hook()
    nc = _build()
    in_maps = prep_inputs(inputs["x"], inputs["weights"], inputs["bias"])
    res = run_bass_kernel_spmd(nc, in_maps, list(range(NCORES)),
                               trace=trace, **kwargs)
    return assemble_output(res.results), res


def kernel(**inputs):
    out, _ = run(inputs)
    return out



# revision 6
# speedup vs baseline: 1.8891x; 1.8891x over previous
"""LocallyConnected2d Trainium2 kernel.

Problem: out[b,o,h,w] = sum_{c,i,j} xpad[b,c,h+i,w+j] * weights[h,w,o,c,i,j] + bias[o,h,w]
  B=32, C=32, O=32, H=W=64, K=3, PAD=1, OH=OW=64.

Sharding: each of the 8 cores owns a band of 8 output rows (OH split), with the
matching 10-row input halo. Weights (the dominant traffic, 144 MiB) split 1/8
per core with zero redundancy.

Device compute: per output location (h,w) the contraction over (c,i,j)=288 is
split into 3 matmuls of K=96 (tap row i fixed, contraction over (c,j)),
accumulated in PSUM. M=o=32, N=b=32. The 4 locations of an ow-group share the
PE array via col-group tile_position packing. All layout transforms (getting
(c,j) onto partitions for both operands) are done host-side in numpy so every
DMA is contiguous.

Host-prepped per-core layouts:
  xp [96, 10*64*32] : [(c,j), r', w, b] = xpad[b, c, 8d+r', w+j]  (x replicated 3x, j-shifted)
  wp [8, 96, 16*384]: [h, (c,j), g, i, w4, o] = weights[8d+h, 4g+w4, o, c, i, j]
  bp [128, 8*16]    : [(w4,o), h, g] = bias[o, 8d+h, 4g+w4]
  op [8, 128, 512]  : [h, (w4,o), g, b]  (output)
"""

import sys

if "/opt/trn_rl_repo" not in sys.path:
    sys.path.insert(0, "/opt/trn_rl_repo")

import numpy as np

B = 32
C = 32
O = 32
H = W = 64
KK = 3
NCORES = 8
RP = H // NCORES      # output rows per core
RIN = RP + KK - 1     # input rows incl halo
P = 96                # contraction partitions (c,j)
NG = W // 4           # ow groups of 4
XGROUPS = [(0, 3), (3, 3), (6, 4)]          # x load groups (start row, nrows)

_built = {}


def _build():
    if "nc" in _built:
        return _built["nc"]
    import concourse.tile as tile
    from concourse.tile import add_dep_helper
    from concourse import bacc, mybir

    nc = bacc.Bacc("TRN2", target_bir_lowering=False, debug=False,
                   num_devices=NCORES)
    f16 = mybir.dt.float16
    xp = nc.dram_tensor("xp", [P, RIN * W * B], f16,
                        kind="ExternalInput")
    wp = nc.dram_tensor("wp", [P, RP * NG * 384], f16,
                        kind="ExternalInput")
    bp = nc.dram_tensor("bp", [128, RP * NG], mybir.dt.float32,
                        kind="ExternalInput")
    op = nc.dram_tensor("op", [RP // 2, 128, 2 * NG * 32], mybir.dt.float32,
                        kind="ExternalOutput")

    f32 = mybir.dt.float32
    HF = NG * 384        # free elems per h row in wp
    with tile.TileContext(nc) as tc:
        with tc.tile_pool(name="xpool", bufs=1) as xpool, \
             tc.tile_pool(name="wpool", bufs=4) as wpool, \
             tc.tile_pool(name="opool", bufs=2) as opool, \
             tc.tile_pool(name="cpool", bufs=1) as cpool, \
             tc.tile_pool(name="ppool", bufs=4, space="PSUM") as ppool:
            loads = []

            def chain(inst):
                # keep at most ~2 load DMAs in flight: each waits for the
                # load two before it to complete
                if len(loads) >= 2:
                    add_dep_helper(inst.ins, loads[-2].ins, sync=True,
                                   reason="load chain")
                loads.append(inst)

            bt = cpool.tile([128, RP * NG], f32, tag="bias")
            chain(nc.sync.dma_start(bt[:], bp.ap()))
            xg = []
            rowtile = {}
            for gi, (r0, nr) in enumerate(XGROUPS):
                t = xpool.tile([P, nr * W * B], f16, tag=f"xg{gi}")
                xg.append(t)
                for r in range(r0, r0 + nr):
                    rowtile[r] = (t, (r - r0) * W * B)

            def load_xg(gi):
                r0, nr = XGROUPS[gi]
                chain(nc.sync.dma_start(
                    xg[gi][:], xp.ap()[:, r0 * W * B:(r0 + nr) * W * B]))

            def load_w(h):
                t = wpool.tile([P, HF], f16, tag="w")
                chain(nc.sync.dma_start(t[:], wp.ap()[:, h * HF:(h + 1) * HF]))
                return t

            load_xg(0)
            wq = [load_w(0), load_w(1)]
            load_xg(1)
            wq.append(load_w(2))
            load_xg(2)
            ot = None
            for h in range(RP):
                wth = wq.pop(0)
                ps = ppool.tile([128, NG * 32], f32, tag="ps")
                for g in range(NG):
                    for w4 in range(4):
                        wo = g * 384 + w4 * 32
                        xo = (4 * g + w4) * 32
                        for i in range(KK):
                            t, base = rowtile[h + i]
                            nc.tensor.matmul(
                                ps[32 * w4:32 * w4 + 32, 32 * g:32 * g + 32],
                                wth[:, wo + i * 128:wo + i * 128 + 32],
                                t[:, base + xo:base + xo + 32],
                                start=(i == 0),
                                stop=(i == KK - 1),
                                tile_position=(0, 32 * w4),
                            )
                if h + 3 < RP:
                    wq.append(load_w(h + 3))
                if h % 2 == 0:
                    ot = opool.tile([128, 2 * NG * 32], f32, tag="o")
                off = (h % 2) * NG * 32
                for g in range(NG):
                    nc.vector.tensor_scalar_add(
                        ot[:, off + 32 * g:off + 32 * g + 32],
                        ps[:, 32 * g:32 * g + 32],
                        bt[:, h * NG + g:h * NG + g + 1],
                    )
                if h % 2 == 1:
                    nc.scalar.dma_start(op.ap()[h // 2], ot[:])
    nc.compile()
    _built["nc"] = nc
    return nc


def prep_inputs(x, weights, bias):
    """Host-side shard + layout prep. Returns list of 8 in_maps."""
    x = np.asarray(x, dtype=np.float32)
    weights = np.asarray(weights, dtype=np.float32)
    bias = np.asarray(bias, dtype=np.float32)
    xpad = np.zeros((B, C, H + 2, W + 2), dtype=np.float32)
    xpad[:, :, 1:H + 1, 1:W + 1] = x
    in_maps = []
    for d in range(NCORES):
        blk = xpad[:, :, RP * d:RP * d + RIN, :]          # [b, c, 10, 66]
        xprep = np.empty((C, KK, RIN, W, B), dtype=np.float16)
        for j in range(KK):
            xprep[:, j] = blk[:, :, :, j:j + W].transpose(1, 2, 3, 0)
        xprep = xprep.reshape(P, RIN * W * B)

        wd = weights[RP * d:RP * d + RP]                  # [8, 64, 32, 32, 3, 3]
        wd = wd.reshape(RP, NG, 4, O, C, KK, KK)          # h, g, w4, o, c, i, j
        wcj = wd.transpose(4, 6, 0, 1, 5, 2, 3)           # c, j, h, g, i, w4, o
        wprep = np.ascontiguousarray(wcj).astype(np.float16).reshape(
            P, RP * NG * 384)

        bd = bias[:, RP * d:RP * d + RP, :].reshape(O, RP, NG, 4)
        bprep = np.ascontiguousarray(bd.transpose(3, 0, 1, 2)).reshape(
            128, RP * NG)                                  # (w4,o), (h,g)
        in_maps.append({"xp": xprep, "wp": wprep, "bp": bprep})
    return in_maps


def assemble_output(results):
    """results: list of 8 dicts with 'op' [4, 128, 1024] -> full [B,O,H,W]."""
    out = np.empty((B, O, H, W), dtype=np.float32)
    for d in range(NCORES):
        arr = np.asarray(results[d]["op"]).reshape(RP // 2, 4, O, 2, NG, B)
        # [ck, w4, o, hh, g, b] -> [b, o, (ck,hh), g, w4]
        out[:, :, RP * d:RP * d + RP, :] = (
            arr.transpose(5, 2, 0, 3, 4, 1).reshape(B, O, RP, W))
    return out


def _ensure_ntff_hook():
    """The agent image's antenv lacks axon_hooks; inject it and register the
    ctypes NTFF hook (same recipe as trn_agent_boot.trn_boot)."""
    try:
        from antenv.axon_hooks import get_axon_ntff_profile_hook  # noqa: F401
        return
    except ImportError:
        pass
    import types
    import ctypes
    import contextlib

    mod = types.ModuleType("antenv.axon_hooks")
    mod._hook = None

    def set_axon_ntff_profile_hook(h):
        mod._hook = h

    def get_axon_ntff_profile_hook():
        return mod._hook

    mod.set_axon_ntff_profile_hook = set_axon_ntff_profile_hook
    mod.get_axon_ntff_profile_hook = get_axon_ntff_profile_hook
    sys.modules["antenv.axon_hooks"] = mod
    import antenv

    antenv.axon_hooks = mod

    so_path = "/opt/axon/libaxon_pjrt.so"
    try:
        lib = ctypes.CDLL(so_path)
    except OSError:
        return
    if not hasattr(lib, "axon_start_nrt_profile"):
        return
    lib.axon_start_nrt_profile.argtypes = [
        ctypes.POINTER(ctypes.c_int64), ctypes.c_size_t]
    lib.axon_start_nrt_profile.restype = ctypes.c_int64
    lib.axon_stop_nrt_profile.argtypes = [ctypes.c_char_p]
    lib.axon_stop_nrt_profile.restype = ctypes.c_int64

    @contextlib.contextmanager
    def _hook(output_dir, device_ids):
        import jax

        jax.devices()
        if device_ids:
            ids = (ctypes.c_int64 * len(device_ids))(*device_ids)
            rc = lib.axon_start_nrt_profile(ids, len(device_ids))
        else:
            rc = lib.axon_start_nrt_profile(None, 0)
        if rc != 0:
            raise RuntimeError(f"axon_start_nrt_profile rc={rc}")
        try:
            yield
        finally:
            n = lib.axon_stop_nrt_profile(str(output_dir).encode())
            print(f"ntff profile: {n} file(s) written to {output_dir}")

    mod.set_axon_ntff_profile_hook(_hook)


def run(inputs, trace=False, **kwargs):
    from concourse.bass_utils import run_bass_kernel_spmd

    if trace:
        _ensure_ntff_hook()
    nc = _build()
    in_maps = prep_inputs(inputs["x"], inputs["weights"], inputs["bias"])
    res = run_bass_kernel_spmd(nc, in_maps, list(range(NCORES)),
                               trace=trace, **kwargs)
    return assemble_output(res.results), res


def kernel(**inputs):
    out, _ = run(inputs)
    return out

